# Initial kernel scaffold
#
"""ASPP pooling head on Trainium2 (Bass/Tile), data-parallel over batch on 8 cores.

Per sample: pooled = mean(x, spatial); y = relu((pooled @ W.T)*bn_scale + bn_shift);
out = broadcast(y, spatial).

Per core (2 samples): 64 MiB of x reads + output writes + 2 MiB weights, all
bounded by the ~436 GB/s per-core DMA/HBM ceiling (measured; matches the
16-engine SDMA aggregate).  Design, from trace analysis:
  - x streamed as 32x [128ch, 4096] f32 tiles (2 MiB) on the sync HWDGE queue
    (the fastest path; SWDGE/gpsimd casts measured ~10% slower).
  - wt is host-prearranged to [128, KCH*COUT] so its load is one contiguous
    DMA on the scalar queue (the naive rearranged AP generated 2048x1KB
    descriptors and 32us of HWDGE sequencer burn, starving the matmuls).
  - spatial sums: VectorE tensor_reduce, with a subset of chunks offloaded to
    ScalarE (activation accum_out) so the reduce chain never lags the stream
    even when the chip clocks down ~20% under load.
  - 16 accumulating PE matmuls per (sample, o-block) with wt = W*bn_scale/4096
    folded on host.
  - broadcast: Relu(psum_bcast + shift) via ScalarE activation (stride-0 src)
    and DVE tensor_scalar for the last sample's second block; output stored as
    bf16 (halves write traffic and the tail; host upcasts; rel err ~3e-3 vs
    the 2e-2 gate).
  - tail: the last two chunks stream as half-tiles with alternating
    ScalarE/DVE reduces so the final reduce completes ~2us after the last
    byte lands; last-sample stores split across both HWDGE queues.
"""

import numpy as np

B, CIN, H, W_SP = 16, 2048, 64, 64
COUT = 256
NCORES = 8
BPC = B // NCORES
SP = H * W_SP
KCH = CIN // 128
NOB = COUT // 128
BN_EPS = 1e-5

# winning config (i3)
SCA_CHUNKS = (1, 4, 7, 10, 13)  # chunks reduced on ScalarE
TAIL_CHUNKS = 2               # last-sample chunks streamed as half-tiles
XIN_BUFS = 9
POOLED_BUFS = 24
HALVES = 2                    # bcast/store pieces per o-block
TAIL_HALVES = 4               # finer pieces for the last sample's stores
BCAST_BUFS = 5
PSUM_BUFS = 4
OUT_BF16 = True
WT_BF16 = True
TAIL_QUARTERS = True
DUAL_ALL = False              # DVE bcast for ob1 only on the last sample

_CACHE = {}


def _build_nc():
    import concourse.bacc as bacc
    import concourse.mybir as mybir
    import concourse.tile as tile

    nc = bacc.Bacc("TRN2", target_bir_lowering=False, debug=False,
                   num_devices=NCORES)
    f32 = mybir.dt.float32
    odt = mybir.dt.bfloat16 if OUT_BF16 else f32
    wdt = mybir.dt.bfloat16 if WT_BF16 else f32
    AT = mybir.ActivationFunctionType
    x = nc.dram_tensor("x", [BPC, CIN, SP], f32, kind="ExternalInput").ap()
    wt = nc.dram_tensor("wt", [128, KCH * COUT], wdt, kind="ExternalInput").ap()
    shift = nc.dram_tensor("shift", [COUT], f32, kind="ExternalInput").ap()
    out = nc.dram_tensor("out", [BPC, COUT, SP], odt,
                         kind="ExternalOutput").ap()

    hsp = SP // HALVES

    with tile.TileContext(nc) as tc, \
         tc.tile_pool(name="consts", bufs=1) as consts, \
         tc.tile_pool(name="xin", bufs=XIN_BUFS) as xin, \
         tc.tile_pool(name="pooled", bufs=POOLED_BUFS) as pooledp, \
         tc.tile_pool(name="psum", bufs=PSUM_BUFS, space="PSUM") as psump, \
         tc.tile_pool(name="bcast", bufs=BCAST_BUFS) as bcastp:

        wt_sb = consts.tile([128, KCH * COUT], wdt)
        shift_sb = consts.tile([128, NOB], f32)
        nc.scalar.dma_start(wt_sb[:], wt)
        nc.scalar.dma_start(shift_sb[:], shift.rearrange("(ob p) -> p ob", p=128))
        zeros_col = consts.tile([128, 1], f32)
        nc.gpsimd.memset(zeros_col[:], 0.0)
        scratch = consts.tile([128, SP], f32)

        for b in range(BPC):
            last = b == BPC - 1
            pss = [psump.tile([128, 1], f32, name=f"ps{ob}", tag=f"ps{ob}")
                   for ob in range(NOB)]

            def reduce_and_mm(src_slice, width, k, scalar_eng, first, stop):
                xt = xin.tile([128, width], f32, name="xt", tag="xt")
                ramp_q = nc.scalar if (b == 0 and k in (1, 3)) else nc.sync
                ramp_q.dma_start(xt[:], src_slice)
                pt = pooledp.tile([128, 1], f32, name="pt", tag="pt")
                if scalar_eng:
                    nc.scalar.activation(scratch[:, :width], xt[:],
                                         AT.Identity, bias=zeros_col[:],
                                         scale=1.0, accum_out=pt[:])
                else:
                    nc.vector.reduce_sum(pt[:], xt[:],
                                         axis=mybir.AxisListType.X)
                if WT_BF16:
                    ptb = pooledp.tile([128, 1], mybir.dt.bfloat16,
                                       name="ptb", tag="ptb")
                    nc.scalar.activation(ptb[:], pt[:], AT.Identity,
                                         scale=1.0)
                    pt = ptb
                for ob in range(NOB):
                    nc.tensor.matmul(
                        pss[ob][:],
                        lhsT=wt_sb[:, k * COUT + ob * 128:
                                   k * COUT + ob * 128 + 128],
                        rhs=pt[:, 0:1],
                        start=first,
                        stop=stop,
                    )

            for k in range(KCH):
                src = x[b, k * 128:(k + 1) * 128, :]
                tail_half = last and k >= KCH - TAIL_CHUNKS
                if TAIL_QUARTERS and last and k == KCH - 1:
                    for d in range(4):
                        reduce_and_mm(src[:, d * (SP // 4):(d + 1) * (SP // 4)],
                                      SP // 4, k, d % 2 == 0,
                                      first=False, stop=(d == 3))
                    continue
                if (b == 0 and k == 0) or tail_half:
                    for d in range(2):
                        sca = tail_half and d == 0
                        reduce_and_mm(src[:, d * (SP // 2):(d + 1) * (SP // 2)],
                                      SP // 2, k, sca,
                                      first=(b == 0 and k == 0 and d == 0),
                                      stop=(k == KCH - 1 and d == 1))
                    continue
                reduce_and_mm(src, SP, k, k in SCA_CHUNKS,
                              first=(k == 0), stop=(k == KCH - 1))

            dual = DUAL_ALL or last
            nh = TAIL_HALVES if last else HALVES
            bhsp = SP // nh
            for h in range(nh):
                for ob in range(NOB):
                    bc = bcastp.tile([128, bhsp], odt, name=f"bc{ob}", tag="bc")
                    src_b = pss[ob][:].broadcast_to([128, bhsp])
                    if dual and ob == 1:
                        nc.vector.tensor_scalar(
                            out=bc[:], in0=src_b,
                            scalar1=shift_sb[:, ob:ob + 1], scalar2=0.0,
                            op0=mybir.AluOpType.add, op1=mybir.AluOpType.max)
                    else:
                        nc.scalar.activation(bc[:], src_b, AT.Relu,
                                             bias=shift_sb[:, ob:ob + 1],
                                             scale=1.0)
                    st_eng = nc.sync if (last and ob == 1) else nc.scalar
                    st_eng.dma_start(
                        out[b, ob * 128:(ob + 1) * 128,
                            h * bhsp:(h + 1) * bhsp], bc[:])

    nc.compile()
    return nc


def _prep_inputs(x, W, gamma, beta, running_mean, running_var):
    scale = np.asarray(gamma, np.float32) / np.sqrt(
        np.asarray(running_var, np.float32) + np.float32(BN_EPS))
    wt = np.ascontiguousarray(
        (np.asarray(W, np.float32) * scale[:, None]).T / np.float32(SP))
    wt_r = np.ascontiguousarray(
        wt.reshape(KCH, 128, COUT).transpose(1, 0, 2).reshape(128, KCH * COUT))
    if WT_BF16:
        import ml_dtypes
        wt_r = wt_r.astype(ml_dtypes.bfloat16)
    shift = (np.asarray(beta, np.float32)
             - np.asarray(running_mean, np.float32) * scale).astype(np.float32)
    xs = np.ascontiguousarray(np.asarray(x, np.float32)).reshape(
        NCORES, BPC, CIN, SP)
    return [{"x": xs[i], "wt": wt_r, "shift": shift} for i in range(NCORES)]


def kernel(x, W, gamma, beta, running_mean, running_var):
    from concourse import bass_utils

    if "nc" not in _CACHE:
        _CACHE["nc"] = _build_nc()
    nc = _CACHE["nc"]
    in_maps = _prep_inputs(x, W, gamma, beta, running_mean, running_var)
    res = bass_utils.run_bass_kernel_spmd(nc, in_maps,
                                          core_ids=list(range(NCORES)))
    outs = [np.asarray(res.results[i]["out"]).astype(np.float32)
            for i in range(NCORES)]
    return np.concatenate(outs, axis=0).reshape(B, COUT, H, W_SP)



# revision 1
# speedup vs baseline: 1.0276x; 1.0276x over previous
"""ASPP pooling head on Trainium2 (Bass/Tile), data-parallel over batch on 8 cores.

Per sample: pooled = mean(x, spatial); y = relu((pooled @ W.T)*bn_scale + bn_shift);
out = broadcast(y, spatial).

Per core (2 samples): 64 MiB of x reads + output writes + 2 MiB weights, all
bounded by the ~436 GB/s per-core DMA/HBM ceiling (measured; matches the
16-engine SDMA aggregate).  Design, from trace analysis:
  - x streamed as 32x [128ch, 4096] f32 tiles (2 MiB) on the sync HWDGE queue
    (the fastest path; SWDGE/gpsimd casts measured ~10% slower).
  - wt is host-prearranged to [128, KCH*COUT] so its load is one contiguous
    DMA on the scalar queue (the naive rearranged AP generated 2048x1KB
    descriptors and 32us of HWDGE sequencer burn, starving the matmuls).
  - spatial sums: VectorE tensor_reduce, with a subset of chunks offloaded to
    ScalarE (activation accum_out) so the reduce chain never lags the stream
    even when the chip clocks down ~20% under load.
  - 16 accumulating PE matmuls per (sample, o-block) with wt = W*bn_scale/4096
    folded on host.
  - broadcast: Relu(psum_bcast + shift) via ScalarE activation (stride-0 src)
    and DVE tensor_scalar for the last sample's second block; output stored as
    bf16 (halves write traffic and the tail; host upcasts; rel err ~3e-3 vs
    the 2e-2 gate).
  - tail: the last two chunks stream as half-tiles with alternating
    ScalarE/DVE reduces so the final reduce completes ~2us after the last
    byte lands; last-sample stores split across both HWDGE queues.
"""

import numpy as np

B, CIN, H, W_SP = 16, 2048, 64, 64
COUT = 256
NCORES = 8
BPC = B // NCORES
SP = H * W_SP
KCH = CIN // 128
NOB = COUT // 128
BN_EPS = 1e-5

# winning config (i3)
SCA_CHUNKS = (1, 4, 7, 10, 13)  # chunks reduced on ScalarE
TAIL_CHUNKS = 2               # last-sample chunks streamed as half-tiles
XIN_BUFS = 9
POOLED_BUFS = 24
HALVES = 2                    # bcast/store pieces per o-block
TAIL_HALVES = 4               # finer pieces for the last sample's stores
BCAST_BUFS = 5
PSUM_BUFS = 4
OUT_BF16 = True
WT_BF16 = True
TAIL_QUARTERS = True
DUAL_ALL = False              # DVE bcast for ob1 only on the last sample

_CACHE = {}


def _build_nc():
    import concourse.bacc as bacc
    import concourse.mybir as mybir
    import concourse.tile as tile

    nc = bacc.Bacc("TRN2", target_bir_lowering=False, debug=False,
                   num_devices=NCORES)
    f32 = mybir.dt.float32
    odt = mybir.dt.bfloat16 if OUT_BF16 else f32
    wdt = mybir.dt.bfloat16 if WT_BF16 else f32
    AT = mybir.ActivationFunctionType
    x = nc.dram_tensor("x", [BPC, CIN, SP], f32, kind="ExternalInput").ap()
    wt = nc.dram_tensor("wt", [128, KCH * COUT], wdt, kind="ExternalInput").ap()
    shift = nc.dram_tensor("shift", [COUT], f32, kind="ExternalInput").ap()
    out = nc.dram_tensor("out", [BPC, COUT, SP], odt,
                         kind="ExternalOutput").ap()

    hsp = SP // HALVES

    with tile.TileContext(nc) as tc, \
         tc.tile_pool(name="consts", bufs=1) as consts, \
         tc.tile_pool(name="xin", bufs=XIN_BUFS) as xin, \
         tc.tile_pool(name="pooled", bufs=POOLED_BUFS) as pooledp, \
         tc.tile_pool(name="psum", bufs=PSUM_BUFS, space="PSUM") as psump, \
         tc.tile_pool(name="bcast", bufs=BCAST_BUFS) as bcastp:

        wt_sb = consts.tile([128, KCH * COUT], wdt)
        shift_sb = consts.tile([128, NOB], f32)
        nc.scalar.dma_start(wt_sb[:], wt)
        nc.scalar.dma_start(shift_sb[:], shift.rearrange("(ob p) -> p ob", p=128))
        zeros_col = consts.tile([128, 1], f32)
        nc.gpsimd.memset(zeros_col[:], 0.0)
        scratch = consts.tile([128, SP], f32)

        for b in range(BPC):
            last = b == BPC - 1
            pss = [psump.tile([128, 1], f32, name=f"ps{ob}", tag=f"ps{ob}")
                   for ob in range(NOB)]

            def reduce_and_mm(src_slice, width, k, scalar_eng, first, stop):
                xt = xin.tile([128, width], f32, name="xt", tag="xt")
                ramp_q = nc.scalar if (b == 0 and k in (1, 3)) else nc.sync
                ramp_q.dma_start(xt[:], src_slice)
                pt = pooledp.tile([128, 1], f32, name="pt", tag="pt")
                if scalar_eng:
                    nc.scalar.activation(scratch[:, :width], xt[:],
                                         AT.Identity, bias=zeros_col[:],
                                         scale=1.0, accum_out=pt[:])
                else:
                    nc.vector.reduce_sum(pt[:], xt[:],
                                         axis=mybir.AxisListType.X)
                if WT_BF16:
                    ptb = pooledp.tile([128, 1], mybir.dt.bfloat16,
                                       name="ptb", tag="ptb")
                    nc.scalar.activation(ptb[:], pt[:], AT.Identity,
                                         scale=1.0)
                    pt = ptb
                for ob in range(NOB):
                    nc.tensor.matmul(
                        pss[ob][:],
                        lhsT=wt_sb[:, k * COUT + ob * 128:
                                   k * COUT + ob * 128 + 128],
                        rhs=pt[:, 0:1],
                        start=first,
                        stop=stop,
                    )

            for k in range(KCH):
                src = x[b, k * 128:(k + 1) * 128, :]
                tail_half = last and k >= KCH - TAIL_CHUNKS
                if TAIL_QUARTERS and last and k == KCH - 1:
                    for d in range(4):
                        reduce_and_mm(src[:, d * (SP // 4):(d + 1) * (SP // 4)],
                                      SP // 4, k, d % 2 == 0,
                                      first=False, stop=(d == 3))
                    continue
                if (b == 0 and k == 0) or tail_half:
                    for d in range(2):
                        sca = tail_half and d == 0
                        reduce_and_mm(src[:, d * (SP // 2):(d + 1) * (SP // 2)],
                                      SP // 2, k, sca,
                                      first=(b == 0 and k == 0 and d == 0),
                                      stop=(k == KCH - 1 and d == 1))
                    continue
                reduce_and_mm(src, SP, k, k in SCA_CHUNKS,
                              first=(k == 0), stop=(k == KCH - 1))

            dual = DUAL_ALL or last
            nh = TAIL_HALVES if last else HALVES
            bhsp = SP // nh
            for h in range(nh):
                for ob in range(NOB):
                    bc = bcastp.tile([128, bhsp], odt, name=f"bc{ob}", tag="bc")
                    src_b = pss[ob][:].broadcast_to([128, bhsp])
                    if dual and ob == 1:
                        nc.vector.tensor_scalar(
                            out=bc[:], in0=src_b,
                            scalar1=shift_sb[:, ob:ob + 1], scalar2=0.0,
                            op0=mybir.AluOpType.add, op1=mybir.AluOpType.max)
                    else:
                        nc.scalar.activation(bc[:], src_b, AT.Relu,
                                             bias=shift_sb[:, ob:ob + 1],
                                             scale=1.0)
                    st_eng = nc.sync if (last and ob == 1) else nc.scalar
                    st_eng.dma_start(
                        out[b, ob * 128:(ob + 1) * 128,
                            h * bhsp:(h + 1) * bhsp], bc[:])

    nc.compile()
    return nc


def _prep_inputs(x, W, gamma, beta, running_mean, running_var):
    scale = np.asarray(gamma, np.float32) / np.sqrt(
        np.asarray(running_var, np.float32) + np.float32(BN_EPS))
    wt = np.ascontiguousarray(
        (np.asarray(W, np.float32) * scale[:, None]).T / np.float32(SP))
    wt_r = np.ascontiguousarray(
        wt.reshape(KCH, 128, COUT).transpose(1, 0, 2).reshape(128, KCH * COUT))
    if WT_BF16:
        import ml_dtypes
        wt_r = wt_r.astype(ml_dtypes.bfloat16)
    shift = (np.asarray(beta, np.float32)
             - np.asarray(running_mean, np.float32) * scale).astype(np.float32)
    xs = np.ascontiguousarray(np.asarray(x, np.float32)).reshape(
        NCORES, BPC, CIN, SP)
    return [{"x": xs[i], "wt": wt_r, "shift": shift} for i in range(NCORES)]


def kernel(x, W, gamma, beta, running_mean, running_var):
    from concourse import bass_utils

    if "nc" not in _CACHE:
        _CACHE["nc"] = _build_nc()
    nc = _CACHE["nc"]
    in_maps = _prep_inputs(x, W, gamma, beta, running_mean, running_var)
    res = bass_utils.run_bass_kernel_spmd(nc, in_maps,
                                          core_ids=list(range(NCORES)))
    outs = [np.asarray(res.results[i]["out"]).astype(np.float32)
            for i in range(NCORES)]
    return np.concatenate(outs, axis=0).reshape(B, COUT, H, W_SP)

